# revision 4
# baseline (speedup 1.0000x reference)
"""Trainium2 Bass kernel for nn_Concatenation_90701119357422.

Computes, for full inputs:
    ret  = mean(ret_feat, axis=1) @ Wp.T + bp          # [B, H]
    out  = concat([h, ret[batch]], -1) @ Wl.T + bl     # [N, H]

Strategy (8 cores, data-parallel over N):
  - out = h @ Wl[:, :H].T + ret2[batch]  where  ret2 = ret @ Wl[:, H:].T + bl
  - host splits h into two contiguous fp16 halves; device DMA-transposes them
    into feature-major SBUF tiles and runs fp16 matmuls w/ fp32 PSUM accumulate
  - ret2 is computed on device from ret_feat (replicated), using a host-folded
    matrix A = (Wp.T/16) @ Wl[:, H:].T and c = bp @ Wl[:, H:].T + bl
  - per-row gather ret2[batch] is a one-hot matmul accumulated into the same
    PSUM tile; the one-hot is built on device from batch values (PE broadcast
    matmul + DVE is_equal); ret2 applied as fp16 hi+lo pair (near-fp32 exact)
"""

import os
import sys

import numpy as np

for _p in ("/opt/trn_rl_repo", "/root/.axon_site/_ro/trn_rl_repo"):
    if os.path.isdir(_p) and _p not in sys.path:
        sys.path.append(_p)

import concourse.bass as bass
import concourse.mybir as mybir
import concourse.tile as tile
from concourse import bacc
from concourse.bass_utils import run_bass_kernel_spmd

N_TOTAL = 262144
B = 64
K = 16
H = 256
R = 512
N_CORES = 8
SHARD = N_TOTAL // N_CORES  # 32768

CHUNK = 1024                 # rows per pipeline chunk
F32 = mybir.dt.float32
F16 = mybir.dt.float16

TRANSPOSE_DMA = "sync"       # engine for h transpose loads
STORE_DMA = "scalar"         # engine for output stores / const loads


def build_program(shard_rows: int = SHARD):
    assert shard_rows % CHUNK == 0
    n_chunks = shard_rows // CHUNK

    nc = bacc.Bacc("TRN2", target_bir_lowering=False, debug=False)

    h16a = nc.dram_tensor("h16a", [shard_rows, 128], F16, kind="ExternalInput").ap()
    h16b = nc.dram_tensor("h16b", [shard_rows, 128], F16, kind="ExternalInput").ap()
    bt = nc.dram_tensor("bt", [1, shard_rows], F16, kind="ExternalInput").ap()
    rf = nc.dram_tensor("rf", [B, K * R], F32, kind="ExternalInput").ap()
    wt16 = nc.dram_tensor("wt16", [H, H], F16, kind="ExternalInput").ap()
    a4 = nc.dram_tensor("a4", [R, H], F32, kind="ExternalInput").ap()
    cvec = nc.dram_tensor("cvec", [1, H], F32, kind="ExternalInput").ap()
    out = nc.dram_tensor("out", [shard_rows, H], F32, kind="ExternalOutput").ap()

    ones_np = np.ones((1, B), dtype=np.float16)
    iota_np = np.arange(B, dtype=np.float32).reshape(B, 1)
    id64_np = np.eye(B, dtype=np.float32)
    ones_dr = nc.inline_tensor(ones_np, "ones64").ap()
    ones32_dr = nc.inline_tensor(np.ones((1, B), dtype=np.float32), "ones64f32").ap()
    iota_dr = nc.inline_tensor(iota_np, "iota64").ap()
    id64_dr = nc.inline_tensor(id64_np, "id64").ap()

    tdma = getattr(nc, TRANSPOSE_DMA)
    sdma = getattr(nc, STORE_DMA)

    with tile.TileContext(nc) as tc:
        with (
            tc.tile_pool(name="const", bufs=1) as cpool,
            tc.tile_pool(name="psum", bufs=1, space="PSUM") as ppool,
            tc.tile_pool(name="ht", bufs=3) as hpool,
            tc.tile_pool(name="oh", bufs=3) as ohpool,
            tc.tile_pool(name="outp", bufs=3) as opool,
        ):
            # ---- constants into SBUF ----
            wt_sb = cpool.tile([128, 2, H], F16)
            sdma.dma_start(wt_sb[:], wt16.rearrange("(kc p) c -> p kc c", p=128))
            a_sb = cpool.tile([128, 4, H], F32)
            sdma.dma_start(a_sb[:], a4.rearrange("(c4 p) n -> p c4 n", p=128))
            cvec_sb = cpool.tile([1, H], F32)
            sdma.dma_start(cvec_sb[:], cvec[:])
            ones_sb = cpool.tile([1, B], F16)
            sdma.dma_start(ones_sb[:], ones_dr[:])
            ones32_sb = cpool.tile([1, B], F32)
            sdma.dma_start(ones32_sb[:], ones32_dr[:])
            iota_sb = cpool.tile([B, 1], F32)
            sdma.dma_start(iota_sb[:], iota_dr[:])
            id64_sb = cpool.tile([B, B], F32)
            sdma.dma_start(id64_sb[:], id64_dr[:])
            bt_sb = cpool.tile([1, shard_rows], F16)
            sdma.dma_start(bt_sb[:], bt[:])
            rf_sb = cpool.tile([B, K * R], F32)
            sdma.dma_start(rf_sb[:], rf[:])

            # ---- preamble: ret2 = (sum_k rf) @ A + c, as fp16 hi+lo ----
            rfv = rf_sb[:].rearrange("b (k r) -> b k r", k=K)
            rsum = cpool.tile([B, R], F32)
            nc.vector.tensor_add(rsum[:], rfv[:, 0], rfv[:, 1])
            for k in range(2, K):
                nc.vector.tensor_add(rsum[:], rsum[:], rfv[:, k])

            r2ps = ppool.tile([B, H], F32, tag="acc")
            for i in range(4):
                tp = ppool.tile([128, B], F32, tag="bc")
                nc.tensor.transpose(
                    tp[:], rsum[:, 128 * i : 128 * (i + 1)], id64_sb[:]
                )
                rst = cpool.tile([128, B], F32, tag=f"rst{i}")
                nc.vector.tensor_copy(rst[:], tp[:])
                nc.tensor.matmul(
                    r2ps[:], rst[:], a_sb[:, i], start=(i == 0), stop=False
                )
            nc.tensor.matmul(r2ps[:], ones32_sb[:], cvec_sb[:], start=False, stop=True)

            ret2hi = cpool.tile([B, H], F16)
            nc.vector.tensor_copy(ret2hi[:], r2ps[:])
            ret2hi32 = cpool.tile([B, H], F32)
            nc.vector.tensor_copy(ret2hi32[:], ret2hi[:])
            ret2lo = cpool.tile([B, H], F16)
            nc.vector.tensor_sub(ret2lo[:], r2ps[:], ret2hi32[:])

            # ---- main loop ----
            for ci in range(n_chunks):
                r0 = ci * CHUNK
                hta = hpool.tile([128, CHUNK], F16, tag="hta")
                tdma.dma_start(
                    out=hta[:], in_=h16a[r0 : r0 + CHUNK, :], transpose=True
                )
                htb = hpool.tile([128, CHUNK], F16, tag="htb")
                tdma.dma_start(
                    out=htb[:], in_=h16b[r0 : r0 + CHUNK, :], transpose=True
                )

                oh = ohpool.tile([B, CHUNK], F16, tag="oh")
                for half in range(CHUNK // 512):
                    rr = r0 + 512 * half
                    bc = ppool.tile([B, 512], F32, tag="bc")
                    nc.tensor.matmul(
                        bc[:],
                        ones_sb[:],
                        bt_sb[0:1, rr : rr + 512],
                        start=True,
                        stop=True,
                    )
                    nc.vector.tensor_scalar(
                        oh[:, 512 * half : 512 * (half + 1)],
                        bc[:],
                        iota_sb[:],
                        None,
                        mybir.AluOpType.is_equal,
                    )

                outsb = opool.tile([128, CHUNK // 128, H], F32, tag="outsb")
                for t in range(CHUNK // 128):
                    ps = ppool.tile([128, H], F32, tag="acc")
                    sl = slice(128 * t, 128 * (t + 1))
                    nc.tensor.matmul(ps[:], hta[:, sl], wt_sb[:, 0], start=True, stop=False)
                    nc.tensor.matmul(ps[:], htb[:, sl], wt_sb[:, 1], start=False, stop=False)
                    nc.tensor.matmul(ps[:], oh[:, sl], ret2hi[:], start=False, stop=False)
                    nc.tensor.matmul(ps[:], oh[:, sl], ret2lo[:], start=False, stop=True)
                    nc.scalar.copy(outsb[:, t], ps[:])

                sdma.dma_start(
                    out=out[r0 : r0 + CHUNK, :].rearrange("(t p) n -> p t n", p=128),
                    in_=outsb[:],
                )

    nc.compile()
    return nc


def prep_inputs(h, ret_feat, batch, Wp, bp, Wl, bl, shard_rows: int = SHARD,
                n_cores: int = N_CORES):
    """Host-side prep: shard + cast. Returns per-core input maps."""
    h = np.asarray(h, dtype=np.float32)
    Wl = np.asarray(Wl, dtype=np.float32)
    Wp = np.asarray(Wp, dtype=np.float32)
    bp = np.asarray(bp, dtype=np.float32)
    bl = np.asarray(bl, dtype=np.float32)
    ret_feat = np.asarray(ret_feat, dtype=np.float32)

    h16a = np.ascontiguousarray(h[:, :128]).astype(np.float16)
    h16b = np.ascontiguousarray(h[:, 128:]).astype(np.float16)
    bt_all = np.asarray(batch).astype(np.float16)

    wt16 = np.ascontiguousarray(Wl[:, :H].T).astype(np.float16)
    wlr_t = Wl[:, H:].astype(np.float64).T  # [R, H]
    a4 = ((Wp.astype(np.float64).T / K) @ wlr_t).astype(np.float32)
    cvec = ((bp.astype(np.float64) @ wlr_t + bl).astype(np.float32)).reshape(1, H)
    rf = np.ascontiguousarray(ret_feat.reshape(B, K * R))

    in_maps = []
    for i in range(n_cores):
        s = slice(i * shard_rows, (i + 1) * shard_rows)
        in_maps.append(
            {
                "h16a": np.ascontiguousarray(h16a[s]),
                "h16b": np.ascontiguousarray(h16b[s]),
                "bt": np.ascontiguousarray(bt_all[s].reshape(1, shard_rows)),
                "rf": rf,
                "wt16": wt16,
                "a4": a4,
                "cvec": cvec,
            }
        )
    return in_maps


_PROGRAM_CACHE = {}


def _get_program(shard_rows: int = SHARD):
    if shard_rows not in _PROGRAM_CACHE:
        _PROGRAM_CACHE[shard_rows] = build_program(shard_rows)
    return _PROGRAM_CACHE[shard_rows]


def kernel(h, ret_feat, batch, Wp, bp, Wl, bl):
    nc = _get_program(SHARD)
    in_maps = prep_inputs(h, ret_feat, batch, Wp, bp, Wl, bl)
    res = run_bass_kernel_spmd(nc, in_maps, list(range(N_CORES)))
    return np.concatenate([res.results[i]["out"] for i in range(N_CORES)], axis=0)


# revision 5
# speedup vs baseline: 1.3299x; 1.3299x over previous
"""Trainium2 Bass kernel for nn_Concatenation_90701119357422.

Computes, for full inputs:
    ret  = mean(ret_feat, axis=1) @ Wp.T + bp          # [B, H]
    out  = concat([h, ret[batch]], -1) @ Wl.T + bl     # [N, H]

Strategy (8 cores, data-parallel over N):
  - out = h @ Wl[:, :H].T + ret2[batch]  where  ret2 = ret @ Wl[:, H:].T + bl
  - host splits h into two contiguous fp16 halves; device DMA-transposes them
    into feature-major SBUF tiles and runs fp16 matmuls w/ fp32 PSUM accumulate
  - ret2 is computed on device from ret_feat (replicated), using a host-folded
    matrix A = (Wp.T/16) @ Wl[:, H:].T and c = bp @ Wl[:, H:].T + bl
  - per-row gather ret2[batch] is a one-hot matmul accumulated into the same
    PSUM tile; the one-hot is built on device from batch values (PE broadcast
    matmul + DVE is_equal); ret2 applied as fp16 hi+lo pair (near-fp32 exact)
"""

import os
import sys

import numpy as np

for _p in ("/opt/trn_rl_repo", "/root/.axon_site/_ro/trn_rl_repo"):
    if os.path.isdir(_p) and _p not in sys.path:
        sys.path.append(_p)

import concourse.bass as bass
import concourse.mybir as mybir
import concourse.tile as tile
from concourse import bacc
from concourse.bass_utils import run_bass_kernel_spmd

N_TOTAL = 262144
B = 64
K = 16
H = 256
R = 512
N_CORES = 8
SHARD = N_TOTAL // N_CORES  # 32768

CHUNK = 1024                 # rows per pipeline chunk
F32 = mybir.dt.float32
F16 = mybir.dt.float16

TRANSPOSE_DMA = "sync"       # engine for h transpose loads
STORE_DMA = "scalar"         # engine for output stores / const loads


def build_program(shard_rows: int = SHARD):
    assert shard_rows % CHUNK == 0
    n_chunks = shard_rows // CHUNK

    nc = bacc.Bacc("TRN2", target_bir_lowering=False, debug=False)

    h16a = nc.dram_tensor("h16a", [shard_rows, 128], F16, kind="ExternalInput").ap()
    h16b = nc.dram_tensor("h16b", [shard_rows, 128], F16, kind="ExternalInput").ap()
    bt = nc.dram_tensor("bt", [1, shard_rows], F16, kind="ExternalInput").ap()
    rf = nc.dram_tensor("rf", [B, K * R], F32, kind="ExternalInput").ap()
    wt16 = nc.dram_tensor("wt16", [H, H], F16, kind="ExternalInput").ap()
    a4 = nc.dram_tensor("a4", [R, H], F32, kind="ExternalInput").ap()
    cvec = nc.dram_tensor("cvec", [1, H], F32, kind="ExternalInput").ap()
    out = nc.dram_tensor("out", [shard_rows, H], F32, kind="ExternalOutput").ap()

    ones_np = np.ones((1, B), dtype=np.float16)
    iota_np = np.arange(B, dtype=np.float32).reshape(B, 1)
    id64_np = np.eye(B, dtype=np.float32)
    ones_dr = nc.inline_tensor(ones_np, "ones64").ap()
    ones32_dr = nc.inline_tensor(np.ones((1, B), dtype=np.float32), "ones64f32").ap()
    iota_dr = nc.inline_tensor(iota_np, "iota64").ap()
    id64_dr = nc.inline_tensor(id64_np, "id64").ap()

    tdma = getattr(nc, TRANSPOSE_DMA)
    sdma = getattr(nc, STORE_DMA)

    with tile.TileContext(nc) as tc:
        with (
            tc.tile_pool(name="const", bufs=1) as cpool,
            tc.tile_pool(name="psum", bufs=1, space="PSUM") as ppool,
            tc.tile_pool(name="ht", bufs=3) as hpool,
            tc.tile_pool(name="oh", bufs=3) as ohpool,
            tc.tile_pool(name="outp", bufs=3) as opool,
        ):
            # ---- constants into SBUF ----
            wt_sb = cpool.tile([128, 2, H], F16)
            sdma.dma_start(wt_sb[:], wt16.rearrange("(kc p) c -> p kc c", p=128))
            a_sb = cpool.tile([128, 4, H], F32)
            sdma.dma_start(a_sb[:], a4.rearrange("(c4 p) n -> p c4 n", p=128))
            cvec_sb = cpool.tile([1, H], F32)
            sdma.dma_start(cvec_sb[:], cvec[:])
            ones_sb = cpool.tile([1, B], F16)
            sdma.dma_start(ones_sb[:], ones_dr[:])
            ones32_sb = cpool.tile([1, B], F32)
            sdma.dma_start(ones32_sb[:], ones32_dr[:])
            iota_sb = cpool.tile([B, 1], F32)
            sdma.dma_start(iota_sb[:], iota_dr[:])
            id64_sb = cpool.tile([B, B], F32)
            sdma.dma_start(id64_sb[:], id64_dr[:])
            bt_sb = cpool.tile([1, shard_rows], F16)
            sdma.dma_start(bt_sb[:], bt[:])
            rf_sb = cpool.tile([B, K * R], F32)
            sdma.dma_start(rf_sb[:], rf[:])

            # ---- preamble: ret2 = (sum_k rf) @ A + c, as fp16 hi+lo ----
            rfv = rf_sb[:].rearrange("b (k r) -> b k r", k=K)
            rsum = cpool.tile([B, R], F32)
            nc.vector.tensor_add(rsum[:], rfv[:, 0], rfv[:, 1])
            for k in range(2, K):
                nc.vector.tensor_add(rsum[:], rsum[:], rfv[:, k])

            r2ps = ppool.tile([B, H], F32, tag="acc", bufs=4)
            for i in range(4):
                tp = ppool.tile([128, B], F32, tag="bc", bufs=2)
                nc.tensor.transpose(
                    tp[:], rsum[:, 128 * i : 128 * (i + 1)], id64_sb[:]
                )
                rst = cpool.tile([128, B], F32, tag=f"rst{i}")
                nc.vector.tensor_copy(rst[:], tp[:])
                nc.tensor.matmul(
                    r2ps[:], rst[:], a_sb[:, i], start=(i == 0), stop=False
                )
            nc.tensor.matmul(r2ps[:], ones32_sb[:], cvec_sb[:], start=False, stop=True)

            ret2hi = cpool.tile([B, H], F16)
            nc.vector.tensor_copy(ret2hi[:], r2ps[:])
            ret2hi32 = cpool.tile([B, H], F32)
            nc.vector.tensor_copy(ret2hi32[:], ret2hi[:])
            ret2lo = cpool.tile([B, H], F16)
            nc.vector.tensor_sub(ret2lo[:], r2ps[:], ret2hi32[:])

            # ---- main loop ----
            for ci in range(n_chunks):
                r0 = ci * CHUNK
                hta = hpool.tile([128, CHUNK], F16, tag="hta")
                tdma.dma_start(
                    out=hta[:], in_=h16a[r0 : r0 + CHUNK, :], transpose=True
                )
                htb = hpool.tile([128, CHUNK], F16, tag="htb")
                tdma.dma_start(
                    out=htb[:], in_=h16b[r0 : r0 + CHUNK, :], transpose=True
                )

                oh = ohpool.tile([B, CHUNK], F16, tag="oh")
                for half in range(CHUNK // 512):
                    rr = r0 + 512 * half
                    bc = ppool.tile([B, 512], F32, tag="bc", bufs=2)
                    nc.tensor.matmul(
                        bc[:],
                        ones_sb[:],
                        bt_sb[0:1, rr : rr + 512],
                        start=True,
                        stop=True,
                    )
                    nc.vector.tensor_scalar(
                        oh[:, 512 * half : 512 * (half + 1)],
                        bc[:],
                        iota_sb[:],
                        None,
                        mybir.AluOpType.is_equal,
                    )

                outsb = opool.tile([128, CHUNK // 128, H], F32, tag="outsb")
                for t in range(CHUNK // 128):
                    ps = ppool.tile([128, H], F32, tag="acc", bufs=4)
                    sl = slice(128 * t, 128 * (t + 1))
                    nc.tensor.matmul(ps[:], hta[:, sl], wt_sb[:, 0], start=True, stop=False)
                    nc.tensor.matmul(ps[:], htb[:, sl], wt_sb[:, 1], start=False, stop=False)
                    nc.tensor.matmul(ps[:], oh[:, sl], ret2hi[:], start=False, stop=False)
                    nc.tensor.matmul(ps[:], oh[:, sl], ret2lo[:], start=False, stop=True)
                    nc.any.tensor_copy(outsb[:, t], ps[:])

                sdma.dma_start(
                    out=out[r0 : r0 + CHUNK, :].rearrange("(t p) n -> p t n", p=128),
                    in_=outsb[:],
                )

    nc.compile()
    return nc


def prep_inputs(h, ret_feat, batch, Wp, bp, Wl, bl, shard_rows: int = SHARD,
                n_cores: int = N_CORES):
    """Host-side prep: shard + cast. Returns per-core input maps."""
    h = np.asarray(h, dtype=np.float32)
    Wl = np.asarray(Wl, dtype=np.float32)
    Wp = np.asarray(Wp, dtype=np.float32)
    bp = np.asarray(bp, dtype=np.float32)
    bl = np.asarray(bl, dtype=np.float32)
    ret_feat = np.asarray(ret_feat, dtype=np.float32)

    h16a = np.ascontiguousarray(h[:, :128]).astype(np.float16)
    h16b = np.ascontiguousarray(h[:, 128:]).astype(np.float16)
    bt_all = np.asarray(batch).astype(np.float16)

    wt16 = np.ascontiguousarray(Wl[:, :H].T).astype(np.float16)
    wlr_t = Wl[:, H:].astype(np.float64).T  # [R, H]
    a4 = ((Wp.astype(np.float64).T / K) @ wlr_t).astype(np.float32)
    cvec = ((bp.astype(np.float64) @ wlr_t + bl).astype(np.float32)).reshape(1, H)
    rf = np.ascontiguousarray(ret_feat.reshape(B, K * R))

    in_maps = []
    for i in range(n_cores):
        s = slice(i * shard_rows, (i + 1) * shard_rows)
        in_maps.append(
            {
                "h16a": np.ascontiguousarray(h16a[s]),
                "h16b": np.ascontiguousarray(h16b[s]),
                "bt": np.ascontiguousarray(bt_all[s].reshape(1, shard_rows)),
                "rf": rf,
                "wt16": wt16,
                "a4": a4,
                "cvec": cvec,
            }
        )
    return in_maps


_PROGRAM_CACHE = {}


def _get_program(shard_rows: int = SHARD):
    if shard_rows not in _PROGRAM_CACHE:
        _PROGRAM_CACHE[shard_rows] = build_program(shard_rows)
    return _PROGRAM_CACHE[shard_rows]


def kernel(h, ret_feat, batch, Wp, bp, Wl, bl):
    nc = _get_program(SHARD)
    in_maps = prep_inputs(h, ret_feat, batch, Wp, bp, Wl, bl)
    res = run_bass_kernel_spmd(nc, in_maps, list(range(N_CORES)))
    return np.concatenate([res.results[i]["out"] for i in range(N_CORES)], axis=0)


# revision 6
# speedup vs baseline: 1.5087x; 1.1345x over previous
"""Trainium2 Bass kernel for nn_Concatenation_90701119357422.

Computes, for full inputs:
    ret  = mean(ret_feat, axis=1) @ Wp.T + bp          # [B, H]
    out  = concat([h, ret[batch]], -1) @ Wl.T + bl     # [N, H]

Strategy (8 cores, data-parallel over N):
  - out = h @ Wl[:, :H].T + ret2[batch]  where  ret2 = ret @ Wl[:, H:].T + bl
  - host casts h to fp16 and pre-transposes it into two feature-major halves
    per core; device runs fp16 matmuls with fp32 PSUM accumulation
  - ret2 is computed on device from ret_feat (replicated), using a host-folded
    matrix A = (Wp.T/16) @ Wl[:, H:].T and c = bp @ Wl[:, H:].T + bl
  - per-row gather ret2[batch] is a one-hot matmul accumulated into the same
    PSUM tile; the one-hot is built on device from batch values (PE broadcast
    matmul + DVE is_equal); ret2 applied as fp16 hi+lo pair (near-fp32 exact)
"""

import os
import sys

import numpy as np

for _p in ("/opt/trn_rl_repo", "/root/.axon_site/_ro/trn_rl_repo"):
    if os.path.isdir(_p) and _p not in sys.path:
        sys.path.append(_p)

import concourse.bass as bass
import concourse.mybir as mybir
import concourse.tile as tile
from concourse import bacc
from concourse.bass_utils import run_bass_kernel_spmd

N_TOTAL = 262144
B = 64
K = 16
H = 256
R = 512
N_CORES = 8
SHARD = N_TOTAL // N_CORES  # 32768

CHUNK = 2048                 # rows per pipeline chunk
F32 = mybir.dt.float32
F16 = mybir.dt.float16


def build_program(shard_rows: int = SHARD):
    assert shard_rows % CHUNK == 0
    n_chunks = shard_rows // CHUNK
    tiles_per_chunk = CHUNK // 128

    nc = bacc.Bacc("TRN2", target_bir_lowering=False, debug=False)

    # feature-major fp16 h halves: hta[k, r] = h[r, k], htb[k, r] = h[r, 128+k]
    hta_d = nc.dram_tensor("hta", [128, shard_rows], F16, kind="ExternalInput").ap()
    htb_d = nc.dram_tensor("htb", [128, shard_rows], F16, kind="ExternalInput").ap()
    bt = nc.dram_tensor("bt", [1, shard_rows], F16, kind="ExternalInput").ap()
    rf = nc.dram_tensor("rf", [B, K * R], F32, kind="ExternalInput").ap()
    wt16 = nc.dram_tensor("wt16", [H, H], F16, kind="ExternalInput").ap()
    a4 = nc.dram_tensor("a4", [R, H], F32, kind="ExternalInput").ap()
    cvec = nc.dram_tensor("cvec", [1, H], F32, kind="ExternalInput").ap()
    out = nc.dram_tensor("out", [shard_rows, H], F32, kind="ExternalOutput").ap()

    ones_dr = nc.inline_tensor(np.ones((1, B), dtype=np.float16), "ones64").ap()
    ones32_dr = nc.inline_tensor(np.ones((1, B), dtype=np.float32), "ones64f32").ap()
    iota_dr = nc.inline_tensor(
        np.arange(B, dtype=np.float32).reshape(B, 1), "iota64"
    ).ap()
    id64_dr = nc.inline_tensor(np.eye(B, dtype=np.float32), "id64").ap()

    with tile.TileContext(nc) as tc:
        with (
            tc.tile_pool(name="const", bufs=1) as cpool,
            tc.tile_pool(name="psum", bufs=1, space="PSUM") as ppool,
            tc.tile_pool(name="ht", bufs=3) as hpool,
            tc.tile_pool(name="oh", bufs=3) as ohpool,
            tc.tile_pool(name="outp", bufs=3) as opool,
        ):
            # ---- constants into SBUF ----
            wt_sb = cpool.tile([128, 2, H], F16)
            nc.scalar.dma_start(wt_sb[:], wt16.rearrange("(kc p) c -> p kc c", p=128))
            a_sb = cpool.tile([128, 4, H], F32)
            nc.scalar.dma_start(a_sb[:], a4.rearrange("(c4 p) n -> p c4 n", p=128))
            cvec_sb = cpool.tile([1, H], F32)
            nc.scalar.dma_start(cvec_sb[:], cvec[:])
            ones_sb = cpool.tile([1, B], F16)
            nc.scalar.dma_start(ones_sb[:], ones_dr[:])
            ones32_sb = cpool.tile([1, B], F32)
            nc.scalar.dma_start(ones32_sb[:], ones32_dr[:])
            iota_sb = cpool.tile([B, 1], F32)
            nc.scalar.dma_start(iota_sb[:], iota_dr[:])
            id64_sb = cpool.tile([B, B], F32)
            nc.scalar.dma_start(id64_sb[:], id64_dr[:])
            rf_sb = cpool.tile([B, K * R], F32)
            nc.scalar.dma_start(rf_sb[:], rf[:])

            # ---- preamble: ret2 = (sum_k rf) @ A + c, as fp16 hi+lo ----
            rfv = rf_sb[:].rearrange("b (k r) -> b k r", k=K)
            rsum = cpool.tile([B, R], F32)
            nc.vector.tensor_add(rsum[:], rfv[:, 0], rfv[:, 1])
            for k in range(2, K):
                nc.vector.tensor_add(rsum[:], rsum[:], rfv[:, k])

            r2ps = ppool.tile([B, H], F32, tag="acc", bufs=4)
            for i in range(4):
                tp = ppool.tile([128, B], F32, tag="bc", bufs=2)
                nc.tensor.transpose(
                    tp[:], rsum[:, 128 * i : 128 * (i + 1)], id64_sb[:]
                )
                rst = cpool.tile([128, B], F32, tag=f"rst{i}")
                nc.vector.tensor_copy(rst[:], tp[:])
                nc.tensor.matmul(
                    r2ps[:], rst[:], a_sb[:, i], start=(i == 0), stop=False
                )
            nc.tensor.matmul(r2ps[:], ones32_sb[:], cvec_sb[:], start=False, stop=True)

            ret2hi = cpool.tile([B, H], F16)
            nc.vector.tensor_copy(ret2hi[:], r2ps[:])
            ret2hi32 = cpool.tile([B, H], F32)
            nc.vector.tensor_copy(ret2hi32[:], ret2hi[:])
            ret2lo = cpool.tile([B, H], F16)
            nc.vector.tensor_sub(ret2lo[:], r2ps[:], ret2hi32[:])

            # ---- main loop ----
            for ci in range(n_chunks):
                r0 = ci * CHUNK
                hta = hpool.tile([128, CHUNK], F16, tag="hta")
                nc.sync.dma_start(out=hta[:], in_=hta_d[:, r0 : r0 + CHUNK])
                htb = hpool.tile([128, CHUNK], F16, tag="htb")
                nc.sync.dma_start(out=htb[:], in_=htb_d[:, r0 : r0 + CHUNK])
                bts = ohpool.tile([1, CHUNK], F16, tag="bts")
                nc.sync.dma_start(out=bts[:], in_=bt[0:1, r0 : r0 + CHUNK])

                oh = ohpool.tile([B, CHUNK], F16, tag="oh")
                for half in range(CHUNK // 512):
                    bc = ppool.tile([B, 512], F32, tag="bc", bufs=2)
                    nc.tensor.matmul(
                        bc[:],
                        ones_sb[:],
                        bts[0:1, 512 * half : 512 * (half + 1)],
                        start=True,
                        stop=True,
                    )
                    nc.vector.tensor_scalar(
                        oh[:, 512 * half : 512 * (half + 1)],
                        bc[:],
                        iota_sb[:],
                        None,
                        mybir.AluOpType.is_equal,
                    )

                outsb = opool.tile([128, tiles_per_chunk, H], F32, tag="outsb")
                for t in range(tiles_per_chunk):
                    ps = ppool.tile([128, H], F32, tag="acc", bufs=4)
                    sl = slice(128 * t, 128 * (t + 1))
                    nc.tensor.matmul(
                        ps[:], hta[:, sl], wt_sb[:, 0], start=True, stop=False
                    )
                    nc.tensor.matmul(
                        ps[:], htb[:, sl], wt_sb[:, 1], start=False, stop=False
                    )
                    nc.tensor.matmul(
                        ps[:], oh[:, sl], ret2hi[:], start=False, stop=False
                    )
                    nc.tensor.matmul(
                        ps[:], oh[:, sl], ret2lo[:], start=False, stop=True
                    )
                    nc.any.tensor_copy(outsb[:, t], ps[:])

                nc.scalar.dma_start(
                    out=out[r0 : r0 + CHUNK, :].rearrange("(t p) n -> p t n", p=128),
                    in_=outsb[:],
                )

    nc.compile()
    return nc


def prep_inputs(h, ret_feat, batch, Wp, bp, Wl, bl, shard_rows: int = SHARD,
                n_cores: int = N_CORES):
    """Host-side prep: shard + cast + pre-transpose h. Returns per-core maps."""
    h = np.asarray(h, dtype=np.float32)
    Wl = np.asarray(Wl, dtype=np.float32)
    Wp = np.asarray(Wp, dtype=np.float32)
    bp = np.asarray(bp, dtype=np.float32)
    bl = np.asarray(bl, dtype=np.float32)
    ret_feat = np.asarray(ret_feat, dtype=np.float32)

    h16 = h.astype(np.float16)
    bt_all = np.asarray(batch).astype(np.float16)

    wt16 = np.ascontiguousarray(Wl[:, :H].T).astype(np.float16)
    wlr_t = Wl[:, H:].astype(np.float64).T  # [R, H]
    a4 = ((Wp.astype(np.float64).T / K) @ wlr_t).astype(np.float32)
    cvec = ((bp.astype(np.float64) @ wlr_t + bl).astype(np.float32)).reshape(1, H)
    rf = np.ascontiguousarray(ret_feat.reshape(B, K * R))

    in_maps = []
    for i in range(n_cores):
        s = slice(i * shard_rows, (i + 1) * shard_rows)
        hs = h16[s]
        in_maps.append(
            {
                "hta": np.ascontiguousarray(hs[:, :128].T),
                "htb": np.ascontiguousarray(hs[:, 128:].T),
                "bt": np.ascontiguousarray(bt_all[s].reshape(1, shard_rows)),
                "rf": rf,
                "wt16": wt16,
                "a4": a4,
                "cvec": cvec,
            }
        )
    return in_maps


_PROGRAM_CACHE = {}


def _get_program(shard_rows: int = SHARD):
    if shard_rows not in _PROGRAM_CACHE:
        _PROGRAM_CACHE[shard_rows] = build_program(shard_rows)
    return _PROGRAM_CACHE[shard_rows]


def kernel(h, ret_feat, batch, Wp, bp, Wl, bl):
    nc = _get_program(SHARD)
    in_maps = prep_inputs(h, ret_feat, batch, Wp, bp, Wl, bl)
    res = run_bass_kernel_spmd(nc, in_maps, list(range(N_CORES)))
    return np.concatenate([res.results[i]["out"] for i in range(N_CORES)], axis=0)


# revision 8
# speedup vs baseline: 2.5843x; 1.7129x over previous
"""Trainium2 Bass kernel for nn_Concatenation_90701119357422.

Computes, for full inputs:
    ret  = mean(ret_feat, axis=1) @ Wp.T + bp          # [B, H]
    out  = concat([h, ret[batch]], -1) @ Wl.T + bl     # [N, H]

Strategy (8 cores, data-parallel over N):
  - out = h @ Wl[:, :H].T + ret2[batch]  where  ret2 = ret @ Wl[:, H:].T + bl
  - host casts h to fp16 and pre-transposes it into two feature-major halves
    per core; device runs fp16 matmuls with fp32 PSUM accumulation
  - ret2 is computed on device from ret_feat (replicated), using a host-folded
    matrix A = (Wp.T/16) @ Wl[:, H:].T and c = bp @ Wl[:, H:].T + bl
  - per-row gather ret2[batch] is a one-hot matmul accumulated into the same
    PSUM tile; the one-hot is built on device from batch values (PE broadcast
    matmul + DVE is_equal); ret2 applied as fp16 hi+lo pair (near-fp32 exact)
"""

import os
import sys

import numpy as np

for _p in ("/opt/trn_rl_repo", "/root/.axon_site/_ro/trn_rl_repo"):
    if os.path.isdir(_p) and _p not in sys.path:
        sys.path.append(_p)

import concourse.bass as bass
import concourse.mybir as mybir
import concourse.tile as tile
from concourse import bacc
from concourse.bass_utils import run_bass_kernel_spmd

N_TOTAL = 262144
B = 64
K = 16
H = 256
R = 512
N_CORES = 8
SHARD = N_TOTAL // N_CORES  # 32768

CHUNK = 2048                 # rows per pipeline chunk
F32 = mybir.dt.float32
F16 = mybir.dt.float16


def build_program(shard_rows: int = SHARD):
    assert shard_rows % CHUNK == 0
    n_chunks = shard_rows // CHUNK
    tiles_per_chunk = CHUNK // 128

    nc = bacc.Bacc("TRN2", target_bir_lowering=False, debug=False)

    # feature-major fp16 h halves: hta[k, r] = h[r, k], htb[k, r] = h[r, 128+k]
    hta_d = nc.dram_tensor("hta", [128, shard_rows], F16, kind="ExternalInput").ap()
    htb_d = nc.dram_tensor("htb", [128, shard_rows], F16, kind="ExternalInput").ap()
    bt = nc.dram_tensor("bt", [1, shard_rows], F16, kind="ExternalInput").ap()
    rf = nc.dram_tensor("rf", [B, K * R], F32, kind="ExternalInput").ap()
    wt16 = nc.dram_tensor("wt16", [H, H], F16, kind="ExternalInput").ap()
    a4 = nc.dram_tensor("a4", [R, H], F32, kind="ExternalInput").ap()
    cvec = nc.dram_tensor("cvec", [1, H], F32, kind="ExternalInput").ap()
    out = nc.dram_tensor("out", [shard_rows, H], F32, kind="ExternalOutput").ap()

    ones_dr = nc.inline_tensor(np.ones((1, B), dtype=np.float16), "ones64").ap()
    ones32_dr = nc.inline_tensor(np.ones((1, B), dtype=np.float32), "ones64f32").ap()
    iota_dr = nc.inline_tensor(
        np.arange(B, dtype=np.float32).reshape(B, 1), "iota64"
    ).ap()
    iota128_dr = nc.inline_tensor(
        np.arange(128, dtype=np.float32).reshape(128, 1), "iota128"
    ).ap()
    id64_dr = nc.inline_tensor(np.eye(B, dtype=np.float32), "id64").ap()

    with tile.TileContext(nc) as tc:
        with (
            tc.tile_pool(name="const", bufs=1) as cpool,
            tc.tile_pool(name="psum", bufs=1, space="PSUM") as ppool,
            tc.tile_pool(name="ht", bufs=3) as hpool,
            tc.tile_pool(name="oh", bufs=3) as ohpool,
            tc.tile_pool(name="outp", bufs=3) as opool,
        ):
            # ---- constants into SBUF ----
            wt_sb = cpool.tile([128, 2, H], F16)
            nc.scalar.dma_start(wt_sb[:], wt16.rearrange("(kc p) c -> p kc c", p=128))
            a_sb = cpool.tile([128, 4, H], F32)
            nc.scalar.dma_start(a_sb[:], a4.rearrange("(c4 p) n -> p c4 n", p=128))
            cvec_sb = cpool.tile([1, H], F32)
            nc.scalar.dma_start(cvec_sb[:], cvec[:])
            ones_sb = cpool.tile([1, B], F16)
            nc.scalar.dma_start(ones_sb[:], ones_dr[:])
            ones32_sb = cpool.tile([1, B], F32)
            nc.scalar.dma_start(ones32_sb[:], ones32_dr[:])
            iota_sb = cpool.tile([B, 1], F32)
            nc.scalar.dma_start(iota_sb[:], iota_dr[:])
            iota128_sb = cpool.tile([128, 1], F32)
            nc.scalar.dma_start(iota128_sb[:], iota128_dr[:])
            id64_sb = cpool.tile([B, B], F32)
            nc.scalar.dma_start(id64_sb[:], id64_dr[:])
            rf_sb = cpool.tile([B, K * R], F32)
            nc.scalar.dma_start(rf_sb[:], rf[:])

            # ---- preamble: ret2 = (sum_k rf) @ A + c, as fp16 hi+lo ----
            rfv = rf_sb[:].rearrange("b (k r) -> b k r", k=K)
            rsum = cpool.tile([B, R], F32)
            nc.vector.tensor_add(rsum[:], rfv[:, 0], rfv[:, 1])
            for k in range(2, K):
                nc.vector.tensor_add(rsum[:], rsum[:], rfv[:, k])

            r2ps = ppool.tile([B, H], F32, tag="acc", bufs=6)
            for i in range(4):
                tp = ppool.tile([128, B], F32, tag="bc", bufs=2)
                nc.tensor.transpose(
                    tp[:], rsum[:, 128 * i : 128 * (i + 1)], id64_sb[:]
                )
                rst = cpool.tile([128, B], F32, tag=f"rst{i}")
                nc.vector.tensor_copy(rst[:], tp[:])
                nc.tensor.matmul(
                    r2ps[:], rst[:], a_sb[:, i], start=(i == 0), stop=False
                )
            nc.tensor.matmul(r2ps[:], ones32_sb[:], cvec_sb[:], start=False, stop=True)

            ret2hi = cpool.tile([128, H], F16)
            nc.gpsimd.memset(ret2hi[:], 0.0)
            nc.vector.tensor_copy(ret2hi[0:B, :], r2ps[:])
            ret2hi32 = cpool.tile([B, H], F32)
            nc.vector.tensor_copy(ret2hi32[:], ret2hi[0:B, :])
            ret2lo = cpool.tile([128, H], F16)
            nc.gpsimd.memset(ret2lo[:], 0.0)
            nc.vector.tensor_sub(ret2lo[0:B, :], r2ps[:], ret2hi32[:])

            # ---- main loop ----
            for ci in range(n_chunks):
                r0 = ci * CHUNK
                hta = hpool.tile([128, CHUNK], F16, tag="hta")
                nc.sync.dma_start(out=hta[:], in_=hta_d[:, r0 : r0 + CHUNK])
                htb = hpool.tile([128, CHUNK], F16, tag="htb")
                nc.sync.dma_start(out=htb[:], in_=htb_d[:, r0 : r0 + CHUNK])
                bts = ohpool.tile([1, CHUNK], F16, tag="bts")
                nc.sync.dma_start(out=bts[:], in_=bt[0:1, r0 : r0 + CHUNK])

                oh = ohpool.tile([128, CHUNK], F16, tag="oh")
                for half in range(CHUNK // 512):
                    hsl = slice(512 * half, 512 * (half + 1))
                    bcb = ohpool.tile([128, 512], F16, tag="bcb", bufs=2)
                    nc.gpsimd.partition_broadcast(bcb[:], bts[0:1, hsl])
                    nc.vector.tensor_scalar(
                        oh[:, hsl],
                        bcb[:],
                        iota128_sb[:],
                        None,
                        mybir.AluOpType.is_equal,
                    )

                outsb = opool.tile([128, tiles_per_chunk, H], F32, tag="outsb")
                for t in range(tiles_per_chunk):
                    ps = ppool.tile([128, H], F32, tag="acc", bufs=6)
                    sl = slice(128 * t, 128 * (t + 1))
                    nc.tensor.matmul(
                        ps[:], hta[:, sl], wt_sb[:, 0], start=True, stop=False
                    )
                    nc.tensor.matmul(
                        ps[:], htb[:, sl], wt_sb[:, 1], start=False, stop=False
                    )
                    nc.tensor.matmul(
                        ps[:], oh[:, sl], ret2hi[:], start=False, stop=False
                    )
                    nc.tensor.matmul(
                        ps[:], oh[:, sl], ret2lo[:], start=False, stop=True
                    )
                    nc.scalar.copy(outsb[:, t], ps[:])

                nc.scalar.dma_start(
                    out=out[r0 : r0 + CHUNK, :].rearrange("(t p) n -> p t n", p=128),
                    in_=outsb[:],
                )

    nc.compile()
    return nc


def prep_inputs(h, ret_feat, batch, Wp, bp, Wl, bl, shard_rows: int = SHARD,
                n_cores: int = N_CORES):
    """Host-side prep: shard + cast + pre-transpose h. Returns per-core maps."""
    h = np.asarray(h, dtype=np.float32)
    Wl = np.asarray(Wl, dtype=np.float32)
    Wp = np.asarray(Wp, dtype=np.float32)
    bp = np.asarray(bp, dtype=np.float32)
    bl = np.asarray(bl, dtype=np.float32)
    ret_feat = np.asarray(ret_feat, dtype=np.float32)

    h16 = h.astype(np.float16)
    bt_all = np.asarray(batch).astype(np.float16)

    wt16 = np.ascontiguousarray(Wl[:, :H].T).astype(np.float16)
    wlr_t = Wl[:, H:].astype(np.float64).T  # [R, H]
    a4 = ((Wp.astype(np.float64).T / K) @ wlr_t).astype(np.float32)
    cvec = ((bp.astype(np.float64) @ wlr_t + bl).astype(np.float32)).reshape(1, H)
    rf = np.ascontiguousarray(ret_feat.reshape(B, K * R))

    in_maps = []
    for i in range(n_cores):
        s = slice(i * shard_rows, (i + 1) * shard_rows)
        hs = h16[s]
        in_maps.append(
            {
                "hta": np.ascontiguousarray(hs[:, :128].T),
                "htb": np.ascontiguousarray(hs[:, 128:].T),
                "bt": np.ascontiguousarray(bt_all[s].reshape(1, shard_rows)),
                "rf": rf,
                "wt16": wt16,
                "a4": a4,
                "cvec": cvec,
            }
        )
    return in_maps


_PROGRAM_CACHE = {}


def _get_program(shard_rows: int = SHARD):
    if shard_rows not in _PROGRAM_CACHE:
        _PROGRAM_CACHE[shard_rows] = build_program(shard_rows)
    return _PROGRAM_CACHE[shard_rows]


def kernel(h, ret_feat, batch, Wp, bp, Wl, bl):
    nc = _get_program(SHARD)
    in_maps = prep_inputs(h, ret_feat, batch, Wp, bp, Wl, bl)
    res = run_bass_kernel_spmd(nc, in_maps, list(range(N_CORES)))
    return np.concatenate([res.results[i]["out"] for i in range(N_CORES)], axis=0)


# revision 9
# speedup vs baseline: 3.1273x; 1.2101x over previous
"""Trainium2 Bass kernel for nn_Concatenation_90701119357422.

Computes, for full inputs:
    ret  = mean(ret_feat, axis=1) @ Wp.T + bp          # [B, H]
    out  = concat([h, ret[batch]], -1) @ Wl.T + bl     # [N, H]

Strategy (8 cores, data-parallel over N):
  - out = h @ Wl[:, :H].T + ret2[batch]  where  ret2 = ret @ Wl[:, H:].T + bl
  - host casts h to fp16 and pre-transposes it into two feature-major halves
    per core; device runs fp16 matmuls with fp32 PSUM accumulation
  - ret2 is computed on device from ret_feat (replicated), using a host-folded
    matrix A = (Wp.T/16) @ Wl[:, H:].T and c = bp @ Wl[:, H:].T + bl
  - per-row gather ret2[batch] is a one-hot matmul accumulated into the same
    PSUM tile; the one-hot is built on device from batch values (PE broadcast
    matmul + DVE is_equal); ret2 applied as fp16 hi+lo pair (near-fp32 exact)
"""

import os
import sys

import numpy as np

for _p in ("/opt/trn_rl_repo", "/root/.axon_site/_ro/trn_rl_repo"):
    if os.path.isdir(_p) and _p not in sys.path:
        sys.path.append(_p)

import concourse.bass as bass
import concourse.mybir as mybir
import concourse.tile as tile
from concourse import bacc
from concourse.bass_utils import run_bass_kernel_spmd

N_TOTAL = 262144
B = 64
K = 16
H = 256
R = 512
N_CORES = 8
SHARD = N_TOTAL // N_CORES  # 32768

CHUNK = 4096                 # rows per pipeline chunk
F32 = mybir.dt.float32
F16 = mybir.dt.float16


def build_program(shard_rows: int = SHARD):
    assert shard_rows % CHUNK == 0
    n_chunks = shard_rows // CHUNK
    tiles_per_chunk = CHUNK // 128

    nc = bacc.Bacc("TRN2", target_bir_lowering=False, debug=False)

    # feature-major fp16 h halves: hta[k, r] = h[r, k], htb[k, r] = h[r, 128+k]
    hta_d = nc.dram_tensor("hta", [128, shard_rows], F16, kind="ExternalInput").ap()
    htb_d = nc.dram_tensor("htb", [128, shard_rows], F16, kind="ExternalInput").ap()
    bt = nc.dram_tensor("bt", [1, shard_rows], F16, kind="ExternalInput").ap()
    wt16 = nc.dram_tensor("wt16", [H, H], F16, kind="ExternalInput").ap()
    r2hi_d = nc.dram_tensor("r2hi", [128, H], F16, kind="ExternalInput").ap()
    r2lo_d = nc.dram_tensor("r2lo", [128, H], F16, kind="ExternalInput").ap()
    out = nc.dram_tensor("out", [shard_rows, H], F32, kind="ExternalOutput").ap()

    iota128_dr = nc.inline_tensor(
        np.arange(128, dtype=np.float32).reshape(128, 1), "iota128"
    ).ap()

    with tile.TileContext(nc) as tc:
        with (
            tc.tile_pool(name="const", bufs=1) as cpool,
            tc.tile_pool(name="psum", bufs=1, space="PSUM") as ppool,
            tc.tile_pool(name="ht", bufs=3) as hpool,
            tc.tile_pool(name="oh", bufs=3) as ohpool,
            tc.tile_pool(name="outp", bufs=3) as opool,
        ):
            # ---- constants into SBUF ----
            wt_sb = cpool.tile([128, 2, H], F16)
            nc.scalar.dma_start(wt_sb[:], wt16.rearrange("(kc p) c -> p kc c", p=128))
            iota128_sb = cpool.tile([128, 1], F32)
            nc.scalar.dma_start(iota128_sb[:], iota128_dr[:])
            ret2hi = cpool.tile([128, H], F16)
            nc.scalar.dma_start(ret2hi[:], r2hi_d[:])
            ret2lo = cpool.tile([128, H], F16)
            nc.scalar.dma_start(ret2lo[:], r2lo_d[:])

            # ---- main loop ----
            for ci in range(n_chunks):
                r0 = ci * CHUNK
                hta = hpool.tile([128, CHUNK], F16, tag="hta")
                nc.sync.dma_start(out=hta[:], in_=hta_d[:, r0 : r0 + CHUNK])
                htb = hpool.tile([128, CHUNK], F16, tag="htb")
                nc.sync.dma_start(out=htb[:], in_=htb_d[:, r0 : r0 + CHUNK])
                bts = ohpool.tile([1, CHUNK], F16, tag="bts")
                nc.sync.dma_start(out=bts[:], in_=bt[0:1, r0 : r0 + CHUNK])

                oh = ohpool.tile([128, CHUNK], F16, tag="oh")
                for half in range(CHUNK // 512):
                    hsl = slice(512 * half, 512 * (half + 1))
                    bcb = ohpool.tile([128, 512], F16, tag="bcb", bufs=2)
                    nc.gpsimd.partition_broadcast(bcb[:], bts[0:1, hsl])
                    nc.vector.tensor_scalar(
                        oh[:, hsl],
                        bcb[:],
                        iota128_sb[:],
                        None,
                        mybir.AluOpType.is_equal,
                    )

                outsb = opool.tile([128, tiles_per_chunk, H], F32, tag="outsb", bufs=2)
                for t in range(tiles_per_chunk):
                    ps = ppool.tile([128, H], F32, tag="acc", bufs=6)
                    sl = slice(128 * t, 128 * (t + 1))
                    nc.tensor.matmul(
                        ps[:], hta[:, sl], wt_sb[:, 0], start=True, stop=False
                    )
                    nc.tensor.matmul(
                        ps[:], htb[:, sl], wt_sb[:, 1], start=False, stop=False
                    )
                    nc.tensor.matmul(
                        ps[:], oh[:, sl], ret2hi[:], start=False, stop=False
                    )
                    nc.tensor.matmul(
                        ps[:], oh[:, sl], ret2lo[:], start=False, stop=True
                    )
                    nc.any.tensor_copy(outsb[:, t], ps[:])

                nc.scalar.dma_start(
                    out=out[r0 : r0 + CHUNK, :].rearrange("(t p) n -> p t n", p=128),
                    in_=outsb[:],
                )

    nc.compile()
    return nc


def prep_inputs(h, ret_feat, batch, Wp, bp, Wl, bl, shard_rows: int = SHARD,
                n_cores: int = N_CORES):
    """Host-side prep: shard + cast + pre-transpose h. Returns per-core maps."""
    h = np.asarray(h, dtype=np.float32)
    Wl = np.asarray(Wl, dtype=np.float32)
    Wp = np.asarray(Wp, dtype=np.float32)
    bp = np.asarray(bp, dtype=np.float32)
    bl = np.asarray(bl, dtype=np.float32)
    ret_feat = np.asarray(ret_feat, dtype=np.float32)

    h16 = h.astype(np.float16)
    bt_all = np.asarray(batch).astype(np.float16)

    wt16 = np.ascontiguousarray(Wl[:, :H].T).astype(np.float16)
    # replicated pooled ret table: ret2 = (mean_k rf) @ Wp.T + bp) @ Wl[:,H:].T + bl
    wlr_t = Wl[:, H:].astype(np.float64).T  # [R, H]
    ret = ret_feat.astype(np.float64).mean(axis=1) @ Wp.astype(np.float64).T + bp
    ret2 = ret @ wlr_t + bl  # [B, H] float64
    r2hi = np.zeros((128, H), dtype=np.float16)
    r2lo = np.zeros((128, H), dtype=np.float16)
    r2hi[:B] = ret2.astype(np.float16)
    r2lo[:B] = (ret2 - r2hi[:B].astype(np.float64)).astype(np.float16)

    in_maps = []
    for i in range(n_cores):
        s = slice(i * shard_rows, (i + 1) * shard_rows)
        hs = h16[s]
        in_maps.append(
            {
                "hta": np.ascontiguousarray(hs[:, :128].T),
                "htb": np.ascontiguousarray(hs[:, 128:].T),
                "bt": np.ascontiguousarray(bt_all[s].reshape(1, shard_rows)),
                "wt16": wt16,
                "r2hi": r2hi,
                "r2lo": r2lo,
            }
        )
    return in_maps


_PROGRAM_CACHE = {}


def _get_program(shard_rows: int = SHARD):
    if shard_rows not in _PROGRAM_CACHE:
        _PROGRAM_CACHE[shard_rows] = build_program(shard_rows)
    return _PROGRAM_CACHE[shard_rows]


def kernel(h, ret_feat, batch, Wp, bp, Wl, bl):
    nc = _get_program(SHARD)
    in_maps = prep_inputs(h, ret_feat, batch, Wp, bp, Wl, bl)
    res = run_bass_kernel_spmd(nc, in_maps, list(range(N_CORES)))
    return np.concatenate([res.results[i]["out"] for i in range(N_CORES)], axis=0)
